# revision 6
# baseline (speedup 1.0000x reference)
"""Multi-level GCN on 8 Trainium2 NeuronCores (Bass/Tile, SPMD).

Algorithm (mathematically identical to the reference, reordered):
  GCN layer:  h' = relu( rsqrt(deg_in) * SUM_e 1[dst_e=.] * rsqrt(deg_out)[src_e]
                          * (X @ W)[src_e]  + b )
  (matmul pushed before the gather: 128-dim rows gathered instead of 256).

Sharding: nodes of each level are range-partitioned across the 8 cores
(row-shard of P1/P2 over the contraction dim). Per core:
  A. XW0 shard = featT_shard.T @ W0 -> bf16 node-major table shard; AllGather.
  B. L0 aggregation over this core's dst tiles (dma_gather rows from the
     table; weighted one-hot built on DVE; S.T@G accumulated on PE);
     per dst-tile: HW1 = H0 @ W1 kept resident in SBUF.
  C. P1 pass: XW1T_partial = HW1.T @ P1_shard (f32r matmuls, N=512),
     streaming P1 from HBM once; AllReduce.
  D. XW1 table: transpose + bf16-cast tiles of the reduced XW1T.
  E. L1 aggregation -> H1 and HW2 = H1 @ W2 resident.
  F. P2 pass (fused): G2T_partial = H1.T @ P2_shard and
     XW2T_partial = HW2.T @ P2_shard share each streamed P2 tile; AllReduce.
  G. emb = G2T.T (full, fp32) + XW2 table build.
  H. L2 aggregation -> out = agg * rsqrt(deg_in2) + b2 (no relu).

Edge lists are sorted by dst on the host and packed per 128-node dst tile
with a fixed per-tile edge budget (cap); padding edges carry weight 0.
"""

import sys

if "/opt/trn_rl_repo" not in sys.path:
    sys.path.insert(0, "/opt/trn_rl_repo")

from contextlib import ExitStack

import ml_dtypes
import numpy as np

import concourse.bacc as bacc
import concourse.bass as bass
import concourse.mybir as mybir
import concourse.tile as tile
from concourse.masks import make_identity

F = mybir.dt.float32
FR = mybir.dt.float32r
BF = mybir.dt.bfloat16
I16 = mybir.dt.int16
I32 = mybir.dt.int32
P = 128
NCORES = 8


# ----------------------------------------------------------------------------
# device program builder
# ----------------------------------------------------------------------------


def build_nc(cfg, debug=False):
    """Build the SPMD Bass program. cfg: dict with
    n0p,n1p,n2p (padded node counts, mult of 128*NCORES), fin, cap0..cap2."""
    n0p, n1p, n2p = cfg["n0p"], cfg["n1p"], cfg["n2p"]
    fin = cfg["fin"]
    caps = [cfg["cap0"], cfg["cap1"], cfg["cap2"]]
    n0c, n1c, n2c = n0p // NCORES, n1p // NCORES, n2p // NCORES
    t0c, t1c, t2c = n0c // P, n1c // P, n2c // P
    kfin = fin // P

    nc = bacc.Bacc(None, target_bir_lowering=False, debug=debug)
    dp = nc.declare_dram_parameter

    featT = dp("featT", [fin, n0c], F, isOutput=False)
    p1 = dp("p1", [n0c, n1p], BF, isOutput=False)
    p2 = dp("p2", [n1c, n2p], BF, isOutput=False)
    w0 = dp("w0", [fin, P], F, isOutput=False)
    w1 = dp("w1", [P, P], F, isOutput=False)
    w2 = dp("w2", [P, P], F, isOutput=False)
    brep = [dp(f"b{l}r", [P, P], F, isOutput=False) for l in range(3)]
    idx = [
        dp(f"idx{l}", [P, t * caps[l] // 16], I16, isOutput=False)
        for l, t in zip(range(3), (t0c, t1c, t2c))
    ]
    dr = [
        dp(f"dr{l}", [P, t * (caps[l] // P)], F, isOutput=False)
        for l, t in zip(range(3), (t0c, t1c, t2c))
    ]
    ws = [
        dp(f"ws{l}", [P, t * (caps[l] // P)], F, isOutput=False)
        for l, t in zip(range(3), (t0c, t1c, t2c))
    ]
    rd = [
        dp(f"rd{l}", [P, t], F, isOutput=False)
        for l, t in zip(range(3), (t0c, t1c, t2c))
    ]
    outy = dp("outy", [n2c, P], F, isOutput=True)
    emb = dp("emb", [n2p, P], F, isOutput=True)

    groups = [list(range(NCORES))]

    with tile.TileContext(nc) as tc, ExitStack() as ctx:
        const = ctx.enter_context(tc.tile_pool(name="const", bufs=1))
        econst = ctx.enter_context(tc.tile_pool(name="econst", bufs=1))
        ftp = ctx.enter_context(tc.tile_pool(name="ftp", bufs=1))
        resid = ctx.enter_context(tc.tile_pool(name="resid", bufs=1))
        gpool = ctx.enter_context(tc.tile_pool(name="gpool", bufs=3))
        spool = ctx.enter_context(tc.tile_pool(name="spool", bufs=4))
        hpool = ctx.enter_context(tc.tile_pool(name="hpool", bufs=3))
        rpool = ctx.enter_context(tc.tile_pool(name="rpool", bufs=12))
        epool = ctx.enter_context(tc.tile_pool(name="epool", bufs=4))
        tpool = ctx.enter_context(tc.tile_pool(name="tpool", bufs=4))
        pss = ctx.enter_context(tc.tile_pool(name="pss", bufs=4, space="PSUM"))
        psl = ctx.enter_context(tc.tile_pool(name="psl", bufs=4, space="PSUM"))
        dram = ctx.enter_context(tc.tile_pool(name="dram", bufs=1, space="DRAM"))

        # ---- constants ------------------------------------------------------
        iota_i = const.tile([P, P], I32)
        nc.gpsimd.iota(iota_i[:], pattern=[[1, P]], base=0, channel_multiplier=0)
        iota_f = const.tile([P, P], F)
        nc.vector.tensor_copy(iota_f[:], iota_i[:])
        ident = const.tile([P, P], F)
        make_identity(nc, ident[:])

        w1_sb = const.tile([P, P], F)
        nc.sync.dma_start(w1_sb[:], w1[:])
        w2_sb = const.tile([P, P], F)
        nc.sync.dma_start(w2_sb[:], w2[:])
        br_sb = []
        for l in range(3):
            b = const.tile([P, P], F, tag=f"br{l}")
            nc.sync.dma_start(b[:], brep[l][:])
            br_sb.append(b)

        idx_sb, dr_sb, ws_sb, rd_sb = [], [], [], []
        for l, t in zip(range(3), (t0c, t1c, t2c)):
            i_ = econst.tile([P, t * caps[l] // 16], I16, tag=f"idx{l}")
            nc.sync.dma_start(i_[:], idx[l][:])
            d_ = econst.tile([P, t * (caps[l] // P)], F, tag=f"dr{l}")
            nc.sync.dma_start(d_[:], dr[l][:])
            s_ = econst.tile([P, t * (caps[l] // P)], F, tag=f"ws{l}")
            nc.sync.dma_start(s_[:], ws[l][:])
            r_ = econst.tile([P, t], F, tag=f"rd{l}")
            nc.sync.dma_start(r_[:], rd[l][:])
            idx_sb.append(i_)
            dr_sb.append(d_)
            ws_sb.append(s_)
            rd_sb.append(r_)

        # ---- stage A: XW0 table shard + AllGather ---------------------------
        ft = []
        for kk in range(kfin):
            f_ = ftp.tile([P, n0c], F, tag=f"ft{kk}")
            nc.sync.dma_start(f_[:], featT[kk * P : (kk + 1) * P, :])
            ft.append(f_)
        w0_sb = []
        for kk in range(kfin):
            w_ = const.tile([P, P], F, tag=f"w0_{kk}")
            nc.sync.dma_start(w_[:], w0[kk * P : (kk + 1) * P, :])
            w0_sb.append(w_)

        xw0_shard = dram.tile([n0c, P], BF)
        xw0_tab = dram.tile([n0p, P], BF)
        CH = 512
        cha = min(CH, n0c)
        for c in range(n0c // cha):
            ps = psl.tile([P, cha], F, tag="psl")
            for kk in range(kfin):
                nc.tensor.matmul(
                    ps[:],
                    w0_sb[kk][:],
                    ft[kk][:, c * cha : (c + 1) * cha],
                    start=(kk == 0),
                    stop=(kk == kfin - 1),
                )
            ev = epool.tile([P, cha], F, tag="ev")
            nc.scalar.activation(ev[:], ps[:], mybir.ActivationFunctionType.Copy)
            for j in range(cha // P):
                tp = pss.tile([P, P], F, tag="pss")
                nc.tensor.matmul(
                    tp[:], ev[:, j * P : (j + 1) * P], ident[:], is_transpose=True
                )
                cb = tpool.tile([P, P], BF, tag="cb")
                nc.vector.tensor_copy(cb[:], tp[:])
                nc.sync.dma_start(
                    xw0_shard[c * cha + j * P : c * cha + (j + 1) * P, :], cb[:]
                )
        nc.gpsimd.collective_compute(
            "AllGather",
            mybir.AluOpType.bypass,
            replica_groups=groups,
            ins=[xw0_shard[:].opt()],
            outs=[xw0_tab[:].opt()],
        )

        # ---- generic aggregation stage --------------------------------------
        GC = 1024  # max idxs per dma_gather the hardware accepts

        def agg_level(l, tcnt, table, bias_sb, relu, per_tile):
            cap = caps[l]
            nb = cap // P
            nchunk = (cap + GC - 1) // GC
            for t in range(tcnt):
                g = gpool.tile([P, nb, P], BF, tag="g")
                for ck in range(nchunk):
                    cw = min(GC, cap - ck * GC)
                    nc.gpsimd.dma_gather(
                        g[:, ck * (GC // P) : ck * (GC // P) + cw // P, :],
                        table[:, :],
                        idx_sb[l][
                            :,
                            (t * cap + ck * GC) // 16 : (t * cap + ck * GC + cw) // 16,
                        ],
                        cw,
                        cw,
                        P,
                    )
                acc = pss.tile([P, P], F, tag="pss")
                for b in range(nb):
                    s = spool.tile([P, P], BF, tag="s")
                    col = t * nb + b
                    nc.vector.tensor_scalar(
                        s[:],
                        iota_f[:],
                        dr_sb[l][:, col : col + 1],
                        ws_sb[l][:, col : col + 1],
                        mybir.AluOpType.is_equal,
                        mybir.AluOpType.mult,
                    )
                    nc.tensor.matmul(
                        acc[:], s[:], g[:, b, :], start=(b == 0), stop=(b == nb - 1)
                    )
                h = hpool.tile([P, P], F, tag="h")
                nc.vector.tensor_scalar(
                    h[:], acc[:], rd_sb[l][:, t : t + 1], None, mybir.AluOpType.mult
                )
                nc.vector.tensor_tensor(h[:], h[:], bias_sb[:], mybir.AluOpType.add)
                if relu:
                    hr = hpool.tile([P, P], F, tag="hr")
                    nc.scalar.activation(
                        hr[:], h[:], mybir.ActivationFunctionType.Relu
                    )
                else:
                    hr = h
                per_tile(hr, t)

        # ---- stage B: L0 agg -> HW1 resident --------------------------------
        hw1_all = resid.tile([P, t0c, P], BF, tag="hw1")

        def l0_tile(hr, t):
            tp = pss.tile([P, P], F, tag="pss")
            nc.tensor.matmul(tp[:], hr[:], ident[:], is_transpose=True)
            h0T = hpool.tile([P, P], F, tag="h0T")
            nc.scalar.activation(h0T[:], tp[:], mybir.ActivationFunctionType.Copy)
            mp = pss.tile([P, P], F, tag="pss")
            nc.tensor.matmul(mp[:], h0T[:], w1_sb[:])
            nc.vector.tensor_copy(hw1_all[:, t, :], mp[:])

        agg_level(0, t0c, xw0_tab, br_sb[0], True, l0_tile)

        # ---- stage C: P1 pass -> XW1T partial -> AllReduce ------------------
        xw1t_part = dram.tile([P, n1p], F)
        xw1t_full = dram.tile([P, n1p], F)
        for c in range(n1p // CH):
            ps = psl.tile([P, CH], F, tag="psl")
            for kt in range(t0c):
                r = rpool.tile([P, CH], BF, tag="r")
                nc.sync.dma_start(
                    r[:], p1[kt * P : (kt + 1) * P, c * CH : (c + 1) * CH]
                )
                nc.tensor.matmul(
                    ps[:],
                    hw1_all[:, kt, :],
                    r[:],
                    start=(kt == 0),
                    stop=(kt == t0c - 1),
                )
            ev = epool.tile([P, CH], F, tag="ev")
            nc.scalar.activation(ev[:], ps[:], mybir.ActivationFunctionType.Copy)
            nc.sync.dma_start(xw1t_part[:, c * CH : (c + 1) * CH], ev[:])
        nc.gpsimd.collective_compute(
            "AllReduce",
            mybir.AluOpType.add,
            replica_groups=groups,
            ins=[xw1t_part[:].opt()],
            outs=[xw1t_full[:].opt()],
        )

        # ---- stage D: XW1 table build ---------------------------------------
        def table_from_featmajor(src_ap, ncols, table, out_f32=None):
            """src [128, ncols] f32 in DRAM -> table [ncols, 128] bf16; also
            optionally write transposed f32 rows to out_f32 dram."""
            for i in range(ncols // P):
                ld = tpool.tile([P, P], F, tag="ld")
                nc.sync.dma_start(ld[:], src_ap[:, i * P : (i + 1) * P])
                tp = pss.tile([P, P], F, tag="pss")
                nc.tensor.matmul(tp[:], ld[:], ident[:], is_transpose=True)
                if table is not None:
                    cb = tpool.tile([P, P], BF, tag="cb")
                    nc.vector.tensor_copy(cb[:], tp[:])
                    nc.sync.dma_start(table[i * P : (i + 1) * P, :], cb[:])
                if out_f32 is not None:
                    cf = tpool.tile([P, P], F, tag="cf")
                    nc.scalar.activation(
                        cf[:], tp[:], mybir.ActivationFunctionType.Copy
                    )
                    nc.sync.dma_start(out_f32[i * P : (i + 1) * P, :], cf[:])

        xw1_tab = dram.tile([n1p, P], BF)
        table_from_featmajor(xw1t_full[:], n1p, xw1_tab)

        # ---- stage E: L1 agg -> H1 + HW2 resident ---------------------------
        h1_all = resid.tile([P, t1c, P], BF, tag="h1")
        hw2_all = resid.tile([P, t1c, P], BF, tag="hw2")

        def l1_tile(hr, t):
            nc.vector.tensor_copy(h1_all[:, t, :], hr[:])
            tp = pss.tile([P, P], F, tag="pss")
            nc.tensor.matmul(tp[:], hr[:], ident[:], is_transpose=True)
            h1T = hpool.tile([P, P], F, tag="h0T")
            nc.scalar.activation(h1T[:], tp[:], mybir.ActivationFunctionType.Copy)
            mp = pss.tile([P, P], F, tag="pss")
            nc.tensor.matmul(mp[:], h1T[:], w2_sb[:])
            nc.vector.tensor_copy(hw2_all[:, t, :], mp[:])

        agg_level(1, t1c, xw1_tab, br_sb[1], True, l1_tile)

        # ---- stage F: fused P2 pass -> [G2T | XW2T] partial -> AllReduce ----
        red_part = dram.tile([2, P, n2p], F)
        red_full = dram.tile([2, P, n2p], F)
        for c in range(n2p // CH):
            psA = psl.tile([P, CH], F, tag="psl")
            psB = psl.tile([P, CH], F, tag="psl")
            for kt in range(t1c):
                r = rpool.tile([P, CH], BF, tag="r")
                nc.sync.dma_start(
                    r[:], p2[kt * P : (kt + 1) * P, c * CH : (c + 1) * CH]
                )
                nc.tensor.matmul(
                    psA[:],
                    h1_all[:, kt, :],
                    r[:],
                    start=(kt == 0),
                    stop=(kt == t1c - 1),
                )
                nc.tensor.matmul(
                    psB[:],
                    hw2_all[:, kt, :],
                    r[:],
                    start=(kt == 0),
                    stop=(kt == t1c - 1),
                )
            for which, ps in ((0, psA), (1, psB)):
                ev = epool.tile([P, CH], F, tag="ev")
                nc.scalar.activation(ev[:], ps[:], mybir.ActivationFunctionType.Copy)
                nc.sync.dma_start(red_part[which, :, c * CH : (c + 1) * CH], ev[:])
        nc.gpsimd.collective_compute(
            "AllReduce",
            mybir.AluOpType.add,
            replica_groups=groups,
            ins=[red_part[:].opt()],
            outs=[red_full[:].opt()],
        )

        # ---- stage G: emb output + XW2 table --------------------------------
        table_from_featmajor(red_full[0], n2p, None, out_f32=emb)
        xw2_tab = dram.tile([n2p, P], BF)
        table_from_featmajor(red_full[1], n2p, xw2_tab)

        # ---- stage H: L2 agg -> outy ----------------------------------------
        def l2_tile(hr, t):
            nc.sync.dma_start(outy[t * P : (t + 1) * P, :], hr[:])

        agg_level(2, t2c, xw2_tab, br_sb[2], False, l2_tile)

    return nc


# ----------------------------------------------------------------------------
# host-side preprocessing
# ----------------------------------------------------------------------------


def _pad_up(x, m):
    return (x + m - 1) // m * m


def _prep_level(src, dst, n, npad, cap_round):
    """Sort edges by dst, pack per 128-node dst tile with fixed cap.
    Returns (cap, idx [T,cap] i16, dstrel [T,cap] f32, wsrc [T,cap] f32,
    rsden [npad] f32) where T = npad//128."""
    src = np.asarray(src).astype(np.int64)
    dst = np.asarray(dst).astype(np.int64)
    d_out = np.maximum(np.bincount(src, minlength=n), 1.0)
    d_in = np.maximum(np.bincount(dst, minlength=n), 1.0)
    rs_out = 1.0 / np.sqrt(d_out.astype(np.float64))
    rs_in = 1.0 / np.sqrt(d_in.astype(np.float64))

    order = np.argsort(dst, kind="stable")
    ds, ss = dst[order], src[order]
    T = npad // P
    tid = ds >> 7
    counts = np.bincount(tid, minlength=T)
    cap = max(cap_round, _pad_up(int(counts.max()), cap_round))
    starts = np.zeros(T, np.int64)
    starts[1:] = np.cumsum(counts)[:-1]
    offs = np.arange(len(ds)) - starts[tid]
    slot = tid * cap + offs
    idx_a = np.zeros(T * cap, np.int16)
    drel_a = np.zeros(T * cap, np.float32)
    wsrc_a = np.zeros(T * cap, np.float32)
    idx_a[slot] = ss.astype(np.int16)
    drel_a[slot] = (ds & 127).astype(np.float32)
    wsrc_a[slot] = rs_out[ss].astype(np.float32)
    rsden = np.ones(npad, np.float32)
    rsden[:n] = rs_in.astype(np.float32)
    return (
        cap,
        idx_a.reshape(T, cap),
        drel_a.reshape(T, cap),
        wsrc_a.reshape(T, cap),
        rsden,
    )


def _idx_wrap(tiles):
    """[Tc, cap] int16 -> [128, Tc*cap//16] device layout (i at [i%16,i//16],
    replicated 8x over partitions)."""
    Tc, cap = tiles.shape
    a = tiles.reshape(Tc, cap // 16, 16)  # [Tc, s, p]
    a = np.transpose(a, (2, 0, 1)).reshape(16, Tc * (cap // 16))
    return np.ascontiguousarray(np.tile(a, (8, 1)))


def _blk_wrap(tiles):
    """[Tc, cap] f32 -> [128, Tc*(cap//128)]: value of edge slot b*128+p of
    tile t at [p, t*nb+b]."""
    Tc, cap = tiles.shape
    nb = cap // P
    a = tiles.reshape(Tc, nb, P)
    a = np.transpose(a, (2, 0, 1)).reshape(P, Tc * nb)
    return np.ascontiguousarray(a)


_CACHE = {}


def _get_nc(cfg_key, cfg):
    if cfg_key not in _CACHE:
        nc = build_nc(cfg)
        nc.compile()
        _CACHE[cfg_key] = nc
    return _CACHE[cfg_key]


def prepare(inputs, cap_round=1024):
    """Full host-side preprocessing: returns (cfg, in_maps, meta)."""
    feats = np.asarray(inputs["features"], np.float32)
    P1 = np.asarray(inputs["P1"], np.float32)
    P2 = np.asarray(inputs["P2"], np.float32)
    n0, fin = feats.shape
    n1, n2 = P1.shape[1], P2.shape[1]
    n0p, n1p, n2p = (_pad_up(x, P * NCORES) for x in (n0, n1, n2))
    n0c, n1c, n2c = n0p // NCORES, n1p // NCORES, n2p // NCORES

    lev = []
    for l, (nl, nlp) in enumerate(((n0, n0p), (n1, n1p), (n2, n2p))):
        lev.append(
            _prep_level(inputs[f"src{l}"], inputs[f"dst{l}"], nl, nlp, cap_round)
        )

    cfg = dict(
        n0p=n0p,
        n1p=n1p,
        n2p=n2p,
        fin=fin,
        cap0=lev[0][0],
        cap1=lev[1][0],
        cap2=lev[2][0],
    )

    featT = np.zeros((fin, n0p), np.float32)
    featT[:, :n0] = feats.T
    P1p = np.zeros((n0p, n1p), np.float32)
    P1p[:n0, :n1] = P1
    P2p = np.zeros((n1p, n2p), np.float32)
    P2p[:n1, :n2] = P2

    def brep(b):
        b = np.asarray(b, np.float32)
        out = np.zeros((P, P), np.float32)
        out[:, : b.shape[0]] = np.tile(b[None, :], (P, 1))
        return out

    b0r, b1r, b2r = (brep(inputs[f"b{l}"]) for l in range(3))
    w0 = np.ascontiguousarray(np.asarray(inputs["W0"], np.float32))
    w1 = np.ascontiguousarray(np.asarray(inputs["W1"], np.float32))
    w2_ = np.asarray(inputs["W2"], np.float32)
    w2 = np.zeros((P, P), np.float32)
    w2[: w2_.shape[0], : w2_.shape[1]] = w2_

    in_maps = []
    for k in range(NCORES):
        m = {
            "featT": np.ascontiguousarray(featT[:, k * n0c : (k + 1) * n0c]),
            "p1": np.ascontiguousarray(P1p[k * n0c : (k + 1) * n0c, :]).astype(ml_dtypes.bfloat16),
            "p2": np.ascontiguousarray(P2p[k * n1c : (k + 1) * n1c, :]).astype(ml_dtypes.bfloat16),
            "w0": w0,
            "w1": w1,
            "w2": w2,
            "b0r": b0r,
            "b1r": b1r,
            "b2r": b2r,
        }
        for l, (tcnt, nlc) in enumerate(((n0p // P, n0c), (n1p // P, n1c), (n2p // P, n2c))):
            cap, idx_t, dr_t, ws_t, rsden = lev[l]
            tc_ = nlc // P
            sl = slice(k * tc_, (k + 1) * tc_)
            m[f"idx{l}"] = _idx_wrap(idx_t[sl])
            m[f"dr{l}"] = _blk_wrap(dr_t[sl])
            m[f"ws{l}"] = _blk_wrap(ws_t[sl])
            rd = rsden[k * nlc : (k + 1) * nlc].reshape(tc_, P).T
            m[f"rd{l}"] = np.ascontiguousarray(rd)
        in_maps.append(m)

    meta = dict(n0=n0, n1=n1, n2=n2, ncls=np.asarray(inputs["W2"]).shape[1])
    return cfg, in_maps, meta


def assemble(results, meta):
    n2, ncls = meta["n2"], meta["ncls"]
    outy = np.concatenate([r["outy"] for r in results], axis=0)
    emb = results[0]["emb"]
    return (
        np.ascontiguousarray(outy[:n2, :ncls]),
        np.ascontiguousarray(emb[:n2, :]),
    )


def kernel(**inputs):
    from concourse.bass_utils import run_bass_kernel_spmd

    cfg, in_maps, meta = prepare(inputs)
    key = tuple(sorted(cfg.items()))
    nc = _get_nc(key, cfg)
    res = run_bass_kernel_spmd(nc, in_maps, list(range(NCORES)))
    return assemble(res.results, meta)
